# revision 6
# baseline (speedup 1.0000x reference)
"""Bass/Trainium2 kernel for nn_Attention_1245540515949.

Reference computation (B=32, T=4096, H=512), fp32 inputs:
    cat    = concat([broadcast(hidden), enc], -1)          # [B,T,2H]
    energy = softmax(cat @ W_attn.T + b_attn, axis=0)      # batch-dim softmax!
    scores = relu(einsum('h,bth->bt', v, energy))[:, None] # [B,1,T]

Strategy: shard T across the 8 cores (the batch softmax stays core-local).
Per core the 512*32 = 16384 (t,b) columns (b inner) are processed in 16
blocks of 1024 columns (32 t each):

  E[h,(t,b)] = W2T.T @ enc + A'[b,h]
      bf16 matmuls, k-chunked 4x128, kc-INNER so each group of 4 accumulates
      into one PSUM bank back-to-back (bank-cycling between accumulating MMs
      costs ~20% PE throughput). A' = hidden@W1.T + b_attn is computed on the
      HOST (exact f32) and added via K=32 "indicator" matmuls; since the
      aprep/ind constants are replicated in all four 32-row groups, the four
      closers of an mc-pair run on four DISTINCT PE row groups concurrently
      (1 N=512 slot instead of 4).
  X   = exp(E)
      ScalarE, one [128,1024] ACT per 2-bank PSUM tile (ACT cost is
      (N+352)/1.2 ns - fewer, larger instructions).
  den[t,h] = sum_b X ; u[h,t] = v[h]/den[t,h]
      DVE segmented reduces (per-mc quarters) + reciprocal_approx_fast + mul
      per mc-half, so most of the den path overlaps the matmul stream and the
      final block's exposed latency is one quarter-reduce.
  scores = u.T @ X
      per pair of blocks, 16 M=32/N=512 matmuls whose four accumulation
      chains (col-groups of one PSUM bank) are interleaved MM-by-MM so they
      run CONCURRENTLY on disjoint PE column groups (~4.5 slots per pair).
      Issued with a 2-pair lag so the PE stream never waits on the den path.
      Valid slots are the block-diagonal [32g + 16*half + jj, 32*jj + b].
  copy + DMA out
      DVE [128,512] copy -> bf16; relu + diagonal extract on HOST.

enc ships as bf16 [H, cols] pre-arranged so each steady-state DMA moves
1 MiB with 8 KiB contiguous per partition (small-descriptor DMAs cap HBM at
~160 GB/s; this layout reaches ~300+ GB/s and minimizes DMA completions,
each of which steals ~1 matmul slot of SBUF bandwidth). Pair 0 is split
into quarter DMAs alternating across both HWDGE queues (sync + scalar) to
cut the pipeline-fill latency. HBM traffic ~17 MiB/core.

Measured (8-core SPMD, NTFF): ~156 us vs 211 us for the previous kernel.
Note the chip has two power states (all-engine clocks 2.4 vs 2.0 GHz); the
same binary measures ~156 us or ~188 us depending on which state the device
is in. Comparisons above are same-state.
"""

import numpy as np

B, T, H = 32, 4096, 512
NCORES = 8
TC = T // NCORES          # 512 t-values per core
P = 128                   # partitions
NCOL = TC * B             # 16384 (t,b) columns per core
NBLK = NCOL // 1024       # 16 blocks of 1024 columns (32 t each)
NPAIR = NBLK // 2         # 8 block-pairs (DMA + scores-PSUM granularity)

_CACHE = {}


def _build_nc():
    import concourse.mybir as mybir
    from concourse.bacc import Bacc
    from concourse.tile import TileContext

    f32 = mybir.dt.float32
    bf16 = mybir.dt.bfloat16
    AF = mybir.ActivationFunctionType
    AX = mybir.AxisListType

    nc = Bacc()

    encb = nc.declare_dram_parameter("encb", [P, NPAIR * 8192], bf16,
                                     isOutput=False)
    w2p = nc.declare_dram_parameter("w2p", [P, 2048], bf16, isOutput=False)
    api = nc.declare_dram_parameter("api", [P, 1024], bf16, isOutput=False)
    vrep = nc.declare_dram_parameter("vrep", [P, P], f32, isOutput=False)
    out = nc.declare_dram_parameter("scores", [P, NPAIR * 512], bf16,
                                    isOutput=True)

    encv = encb.rearrange("p (pr kc j n) -> p pr kc j n", pr=NPAIR, kc=4, j=2)

    with TileContext(nc) as tc:
        with (
            tc.tile_pool(name="consts", bufs=1) as consts,
            tc.tile_pool(name="enc", bufs=NPAIR - 1) as encp,
            tc.tile_pool(name="xs", bufs=6) as xp,
            tc.tile_pool(name="dens", bufs=6) as dp,
            tc.tile_pool(name="us", bufs=6) as up,
            tc.tile_pool(name="scb", bufs=3) as scb,
            tc.tile_pool(name="eps", bufs=3, space="PSUM") as eps,
            tc.tile_pool(name="scps", bufs=2, space="PSUM") as scps,
        ):
            # ---- constants into SBUF. Startup latency matters: the first
            #      matmuls gate on w2[kc01] + the j=0 half of enc pair 0.
            #      Pair 0 is stored (j, kc, n) in DRAM (unlike the steady
            #      (kc, j, n) pairs) so each j-half is ONE contiguous
            #      [P,4096] DMA (8 KiB/partition -> full HBM rate) instead
            #      of four small ones; w2 is split in two so the first
            #      accumulation group doesn't gate on all of it. ----
            w2_sb = consts.tile([P, 2048], bf16, name="w2p")
            e0_sb = [consts.tile([P, 4096], bf16, name=f"enc0_{j}")
                     for j in range(2)]
            api_sb = consts.tile([P, 1024], bf16, name="api")
            vrep_sb = consts.tile([P, P], f32, name="vrep")

            nc.sync.dma_start(out=e0_sb[0], in_=encb[:, 0:4096])
            nc.sync.dma_start(out=vrep_sb, in_=vrep[:, :])
            nc.scalar.dma_start(out=w2_sb[:, 0:1024], in_=w2p[:, 0:1024])
            nc.scalar.dma_start(out=w2_sb[:, 1024:2048], in_=w2p[:, 1024:2048])
            nc.scalar.dma_start(out=api_sb, in_=api[:, :])
            nc.scalar.dma_start(out=e0_sb[1], in_=encb[:, 4096:8192])
            e0v = [t_.rearrange("p (kc n) -> p kc n", kc=4) for t_ in e0_sb]
            # prewarm the exp table set so ACT_TABLE_LOAD overlaps the
            # enc prefetch instead of stalling the first tile
            warm = consts.tile([1, 1], bf16, name="actwarm")
            nc.scalar.activation(out=warm, in_=w2_sb[0:1, 0:1], func=AF.Exp)

            # ---- main loop (scores lag 2 blocks behind the E/X pipeline
            #      so the PE instruction stream never stalls on den/u) ----
            x_hist = [None] * NBLK
            u_hist = [None] * NBLK
            sc_ps = None
            for it in range(NBLK + 3):
                if it < NBLK:
                    blk = it
                    pair, j = blk // 2, blk % 2
                    if j == 0 and pair > 0:
                        etile = encp.tile([P, 8192], bf16, tag="enc")
                        for k2 in range(2):
                            nc.sync.dma_start(
                                out=etile[:, k2 * 4096:(k2 + 1) * 4096],
                                in_=encb[:, pair * 8192 + k2 * 4096:
                                         pair * 8192 + (k2 + 1) * 4096],
                            )
                        ev = etile.rearrange(
                            "p (kc j n) -> p kc j n", kc=4, j=2)
                    if pair == 0:
                        # pair-0 DRAM layout is (j, kc, n)
                        eb = [e0v[j][:, kc] for kc in range(4)]
                    else:
                        eb = [ev[:, kc, j] for kc in range(4)]

                    x_all = xp.tile([P, 4096], bf16, tag="x")
                    x_hist[blk] = x_all
                    # mc-pair structure: 4 consecutive kc-MMs accumulate into
                    # ONE psum bank (avoids per-MM bank cycling, a PE
                    # micro-idle trap), and the K=32 A'-closers of two mc
                    # tiles are issued adjacently so their disjoint PE row
                    # groups overlap.
                    for mp in range(2):
                        mcs = (2 * mp, 2 * mp + 1)
                        ep_of = {}
                        for mc in mcs:
                            ep = eps.tile([P, 1024], f32, tag="e")
                            ep_of[mc] = ep
                            for half in range(2):
                                for kc in range(4):
                                    nc.tensor.matmul(
                                        out=ep[:, half * 512:
                                               (half + 1) * 512],
                                        lhsT=w2_sb[:, kc * 512 + mc * P:
                                                    kc * 512 + (mc + 1) * P],
                                        rhs=eb[kc][:, half * 512:
                                                   (half + 1) * 512],
                                        start=(kc == 0), stop=False,
                                    )
                        # the aprep/ind constants are replicated in all 4
                        # 32-row groups, so each of the 4 closers of this
                        # mc-pair can use a DISTINCT PE row group -> all four
                        # run concurrently (1 N=512 slot instead of 4)
                        for half in range(2):
                            for mc in mcs:
                                rg = (mc + 2 * half) % 4
                                nc.tensor.matmul(
                                    out=ep_of[mc][:, half * 512:
                                                  (half + 1) * 512],
                                    lhsT=api_sb[32 * rg:32 * (rg + 1),
                                                mc * P:(mc + 1) * P],
                                    rhs=api_sb[32 * rg:32 * (rg + 1),
                                               512:1024],
                                    start=False, stop=True,
                                    tile_position=(32 * rg, 0),
                                )
                        last = blk == NBLK - 1
                        for mc in mcs:
                            if last:
                                # final block: halve ACT granularity so the
                                # closing den chain starts ~0.6us sooner
                                for hf in range(2):
                                    nc.scalar.activation(
                                        out=x_all[:, mc * 1024 + hf * 512:
                                                  mc * 1024 + hf * 512 + 512],
                                        in_=ep_of[mc][:, hf * 512:
                                                      hf * 512 + 512],
                                        func=AF.Exp,
                                    )
                            else:
                                nc.scalar.activation(
                                    out=x_all[:, mc * 1024:(mc + 1) * 1024],
                                    in_=ep_of[mc], func=AF.Exp,
                                )

                    # den path per mc-half (the low half only needs the
                    # first mc-pair's exps, so it overlaps the second pair's
                    # matmuls and shortens the final-block tail)
                    x3 = x_all.rearrange("p (mt b) -> p mt b", b=32)
                    us = []
                    nred = 4 if last else 2
                    for hh in range(2):
                        den = dp.tile([P, 64], f32, tag=f"den{hh}")
                        for q in range(nred):
                            w = 64 // nred
                            nc.vector.reduce_sum(
                                out=den[:, q * w:(q + 1) * w],
                                in_=x3[:, hh * 64 + q * w:
                                       hh * 64 + (q + 1) * w, :],
                                axis=AX.X)
                        rden = dp.tile([P, 64], f32, tag=f"rden{hh}")
                        u = up.tile([P, 64], bf16, tag=f"u{hh}")
                        if last:
                            for h2 in range(2):
                                sl = slice(h2 * 32, (h2 + 1) * 32)
                                nc.vector.reciprocal_approx_fast(
                                    out=rden[:, sl], in_=den[:, sl])
                                nc.vector.tensor_mul(
                                    out=u[:, sl], in0=rden[:, sl],
                                    in1=vrep_sb[:, hh * 64 + h2 * 32:
                                                hh * 64 + (h2 + 1) * 32])
                        else:
                            nc.vector.reciprocal_approx_fast(
                                out=rden, in_=den)
                            nc.vector.tensor_mul(
                                out=u, in0=rden,
                                in1=vrep_sb[:, hh * 64:(hh + 1) * 64])
                        us.append(u)
                    u_hist[blk] = us

                # scores for pair p at it == 2p+4: all 4 col-group chains
                # (g = 2*sj + half) interleaved MM-by-MM so they run
                # CONCURRENTLY on disjoint PE column groups -- 16 matmuls in
                # ~4-5 N=512 slots instead of 16. Valid slots are
                # out[32*g + 16*half + jj, 32*jj + b].
                if it >= 4 and (it - 4) % 2 == 0 and (it - 4) // 2 < NPAIR:
                    spair = (it - 4) // 2
                    sc_ps = scps.tile([P, 512], f32, tag="sc")
                    for mc in range(4):
                        for g in range(4):
                            sj, half = divmod(g, 2)
                            sblk = 2 * spair + sj
                            nc.tensor.matmul(
                                out=sc_ps[32 * g:32 * (g + 1), :],
                                lhsT=u_hist[sblk][mc // 2][
                                    :, (mc % 2) * 32:(mc % 2) * 32 + 32],
                                rhs=x_hist[sblk][:, mc * 1024 + half * 512:
                                                mc * 1024 + half * 512 + 512],
                                start=(mc == 0), stop=(mc == 3),
                                tile_position=(0, 32 * g),
                            )
                    ssb = scb.tile([P, 512], bf16, tag="ssb")
                    nc.vector.tensor_copy(out=ssb, in_=sc_ps)
                    nc.sync.dma_start(
                        out=out[:, spair * 512:(spair + 1) * 512],
                        in_=ssb,
                    )

    nc.compile()
    return nc


def _prep_inputs(hidden, encoder_outputs, W_attn, b_attn, v):
    """Host-side shard + layout prep. Returns in_maps for the 8 cores."""
    import ml_dtypes
    bf16 = ml_dtypes.bfloat16

    hidden = np.asarray(hidden, dtype=np.float32)
    enc = np.asarray(encoder_outputs, dtype=np.float32)
    W = np.asarray(W_attn, dtype=np.float32)
    b = np.asarray(b_attn, dtype=np.float32)
    v = np.asarray(v, dtype=np.float32)

    w2t = np.ascontiguousarray(W[:, H:].T)                   # [h_in, h_out]
    w2p = np.ascontiguousarray(
        w2t.reshape(4, P, H).transpose(1, 0, 2).reshape(P, 2048)
    ).astype(bf16)
    # A' = hidden @ W1.T + b_attn, exact on host, replicated to the 4
    # 32-row groups used by the indicator matmuls
    apr = hidden @ W[:, :H].T + b[None, :]                   # [B, H]
    aprep = np.tile(apr, (4, 1))                             # [128, 512]
    ind = np.tile(np.eye(B, dtype=np.float32), (4, 512 // B))
    api = np.concatenate([aprep, ind], axis=1).astype(bf16)  # [128, 1024]
    vcol = np.ascontiguousarray(v.reshape(4, P).T)           # [P, 4] f32
    vrep = np.repeat(vcol, 32, axis=1).astype(np.float32)    # [P, 128]

    in_maps = []
    for c in range(NCORES):
        shard = enc[c * TC:(c + 1) * TC]                     # [TC, B, H]
        encT = shard.reshape(NCOL, H).T                      # [H, NCOL]
        encb = np.ascontiguousarray(
            encT.reshape(4, P, NPAIR, 2, 1024)
                .transpose(1, 2, 0, 3, 4).reshape(P, NPAIR * 8192)
        )
        # pair 0 is stored (j, kc, n) so each j-half is one contiguous DMA
        encb[:, :8192] = np.ascontiguousarray(
            encb[:, :8192].reshape(P, 4, 2, 1024).transpose(0, 2, 1, 3)
            .reshape(P, 8192))
        in_maps.append({
            "encb": encb.astype(bf16), "w2p": w2p, "api": api, "vrep": vrep,
        })
    return in_maps


def _assemble(results):
    """results: per-core dicts with 'scores' [128, NPAIR*512] bf16.

    Column layout: col = pair*512 + 32*jj + b. Valid rows per quarter q
    (t = 64*pair + 16*q + jj): q=0 -> row jj, q=1 -> 48+jj, q=2 -> 64+jj,
    q=3 -> 112+jj.
    """
    rowbase = (0, 48, 64, 112)
    out = np.empty((B, 1, T), np.float32)
    for c in range(NCORES):
        s = np.asarray(results[c]["scores"], dtype=np.float32)
        s4 = s.reshape(P, NPAIR, 16, B)                      # [row,pair,jj,b]
        for q in range(4):
            for jj in range(16):
                vals = s4[rowbase[q] + jj, :, jj, :]         # [pair, b]
                t0 = c * TC + 16 * q + jj
                out[:, 0, t0:t0 + 64 * NPAIR:64] = np.maximum(vals, 0.0).T
    return out


def run(in_maps, trace=False, **kw):
    from concourse.bass_utils import run_bass_kernel_spmd

    if "nc" not in _CACHE:
        _CACHE["nc"] = _build_nc()
    nc = _CACHE["nc"]
    return run_bass_kernel_spmd(
        nc, in_maps, list(range(NCORES)), trace=trace, **kw
    )


def kernel(hidden, encoder_outputs, W_attn, b_attn, v):
    in_maps = _prep_inputs(hidden, encoder_outputs, W_attn, b_attn, v)
    br = run(in_maps)
    return _assemble(br.results)



# revision 8
# speedup vs baseline: 1.1870x; 1.1870x over previous
"""Bass/Trainium2 kernel for nn_Attention_1245540515949.

Reference computation (B=32, T=4096, H=512), fp32 inputs:
    cat    = concat([broadcast(hidden), enc], -1)          # [B,T,2H]
    energy = softmax(cat @ W_attn.T + b_attn, axis=0)      # batch-dim softmax!
    scores = relu(einsum('h,bth->bt', v, energy))[:, None] # [B,1,T]

Strategy: shard T across the 8 cores (the batch softmax stays core-local).
Per core the 512*32 = 16384 (t,b) columns (b inner) are processed in 16
blocks of 1024 columns (32 t each):

  E[h,(t,b)] = W2T.T @ enc + A'[b,h]
      Mixed-precision contraction over K=512 (4 chunks of 128): the first
      NKC8 chunks run as fp8-e4m3 DoubleRow matmuls (2x PE rate) using a
      dual-scale residual trick -- each DoubleRow computes
      W8.enc_hi + (W8/16).(16*(enc-enc_hi)) so the enc quantization error
      cancels to ~0.13% (better than bf16); only the fp8 W error remains,
      shrunk by per-row scale search. Remaining chunks are bf16. All
      contributions are pre-scaled per E-row by s_h (absorbed into W8, the
      bf16 W chunks and A'); the exp activation de-scales via its
      per-partition scale operand. DoubleRow and bf16 matmuls alternate so
      the 256-col fp8 LDWEIGHTS hide under the longer bf16 matmuls.
      A' = hidden@W1.T + b_attn is computed on the HOST (exact f32) and
      added via K=32 "indicator" matmuls; the four closers of an mc-pair
      run on four DISTINCT PE row groups concurrently.
  X   = exp(E * (1/s_h))
      ScalarE, one [128,1024] ACT per 2-bank PSUM tile, fp16 out.
  den[t,h] = sum_b X ; u[h,t] = v[h]/den[t,h]
      DVE segmented reduces with fp16 in/out (2-byte dtypes enable the DVE
      2x port mode) + tiny casts to f32 for reciprocal_approx_fast.
  scores = u.T @ X
      per pair of blocks, 16 M=32/N=512 fp16 matmuls whose four
      accumulation chains are interleaved so they run concurrently on
      disjoint PE column groups. Issued with a 2-pair lag.
  copy + DMA out (fp16); relu + diagonal extract on HOST.

enc ships as (hi,res) fp8 pairs for the fp8 chunks and bf16 for the rest
(16 KiB/partition/pair either way). Pair 0 is stored j-major and split
across both HWDGE queues so block 0 is data-complete early. The final
block's ACT/reduce/reciprocal run at half granularity to shorten the
closing dependency chain.
"""

import numpy as np

B, T, H = 32, 4096, 512
NCORES = 8
TC = T // NCORES          # 512 t-values per core
P = 128                   # partitions
NCOL = TC * B             # 16384 (t,b) columns per core
NBLK = NCOL // 1024       # 16 blocks of 1024 columns (32 t each)
NPAIR = NBLK // 2         # 8 block-pairs (DMA + scores-PSUM granularity)

NKC8 = 2                  # k-chunks (of 4) done in fp8 DoubleRow
NKCB = 4 - NKC8           # bf16 k-chunks
SGRID = np.linspace(20.0, 60.0, 41)

_CACHE = {}


def _build_nc():
    import concourse.mybir as mybir
    from concourse.bacc import Bacc
    from concourse.tile import TileContext

    f32 = mybir.dt.float32
    bf16 = mybir.dt.bfloat16
    fp16 = mybir.dt.float16
    f8 = mybir.dt.float8e4
    AF = mybir.ActivationFunctionType
    AX = mybir.AxisListType
    DR = mybir.MatmulPerfMode.DoubleRow

    nc = Bacc()

    # per pair: enc8 (kc8, j, plane, 1024) fp8 ; encbf (kcb, j, 1024) bf16
    enc8 = nc.declare_dram_parameter(
        "enc8", [P, NPAIR * NKC8 * 4096], f8, isOutput=False)
    encbf = nc.declare_dram_parameter(
        "encbf", [P, NPAIR * NKCB * 2048], bf16, isOutput=False)
    w8p = nc.declare_dram_parameter(
        "w8p", [P, NKC8 * 1024], f8, isOutput=False)    # (kc8, mc, plane, m)
    w2p = nc.declare_dram_parameter(
        "w2p", [P, NKCB * 512], bf16, isOutput=False)   # (kcb, m512)
    api = nc.declare_dram_parameter("api", [P, 1024], bf16, isOutput=False)
    vrep = nc.declare_dram_parameter("vrep", [P, P], f32, isOutput=False)
    sinv = nc.declare_dram_parameter("sinv", [P, 4], f32, isOutput=False)
    out = nc.declare_dram_parameter("scores", [P, NPAIR * 512], fp16,
                                    isOutput=True)

    e8v_ = enc8.rearrange("p (pr kc j pl n) -> p pr kc j pl n",
                          pr=NPAIR, kc=NKC8, j=2, pl=2)
    ebv_ = encbf.rearrange("p (pr kc j n) -> p pr kc j n",
                           pr=NPAIR, kc=NKCB, j=2)

    with TileContext(nc) as tc:
        with (
            tc.tile_pool(name="consts", bufs=1) as consts,
            tc.tile_pool(name="enc8p", bufs=NPAIR - 1) as enc8p,
            tc.tile_pool(name="encbp", bufs=NPAIR - 1) as encbp,
            tc.tile_pool(name="xs", bufs=6) as xp,
            tc.tile_pool(name="dens", bufs=6) as dp,
            tc.tile_pool(name="us", bufs=6) as up,
            tc.tile_pool(name="scb", bufs=3) as scb,
            tc.tile_pool(name="eps", bufs=3, space="PSUM") as eps,
            tc.tile_pool(name="scps", bufs=2, space="PSUM") as scps,
        ):
            # ---- constants into SBUF; critical first-block bytes (weights
            #      + pair-0 j0) balanced across both HWDGE queues.
            #      Pair 0 DRAM layout is j-major: enc8 (j, kc8, pl, n),
            #      encbf (j, kcb, n). ----
            w8_sb = consts.tile([P, NKC8 * 1024], f8, name="w8p")
            w2_sb = consts.tile([P, NKCB * 512], bf16, name="w2p")
            e80 = [consts.tile([P, NKC8 * 2048], f8, name=f"e80_{j}")
                   for j in range(2)]
            eb0 = [consts.tile([P, NKCB * 1024], bf16, name=f"eb0_{j}")
                   for j in range(2)]
            api_sb = consts.tile([P, 1024], bf16, name="api")
            vrep_sb = consts.tile([P, P], f32, name="vrep")
            sinv_sb = consts.tile([P, 4], f32, name="sinv")

            E8C = NKC8 * 2048   # fp8 bytes/partition per j of a pair
            EBC = NKCB * 1024   # bf16 cols/partition per j of a pair
            nc.sync.dma_start(out=e80[0], in_=enc8[:, 0:E8C])
            nc.sync.dma_start(out=eb0[0], in_=encbf[:, 0:EBC])
            nc.sync.dma_start(out=api_sb, in_=api[:, :])
            nc.sync.dma_start(out=e80[1], in_=enc8[:, E8C:2 * E8C])
            nc.sync.dma_start(out=eb0[1], in_=encbf[:, EBC:2 * EBC])
            nc.sync.dma_start(out=vrep_sb, in_=vrep[:, :])
            nc.scalar.dma_start(out=w8_sb, in_=w8p[:, :])
            nc.scalar.dma_start(out=w2_sb, in_=w2p[:, :])
            nc.scalar.dma_start(out=sinv_sb, in_=sinv[:, :])
            w8v = w8_sb.rearrange("p (kc mc pl m) -> p kc mc pl m",
                                  kc=NKC8, mc=4, pl=2)
            e80v = [t_.rearrange("p (kc pl n) -> p kc pl n", kc=NKC8, pl=2)
                    for t_ in e80]
            eb0v = [t_.rearrange("p (kc n) -> p kc n", kc=NKCB)
                    for t_ in eb0]

            # prewarm the exp table set so ACT_TABLE_LOAD overlaps the
            # enc prefetch instead of stalling the first tile
            warm = consts.tile([1, 1], bf16, name="actwarm")
            nc.scalar.activation(out=warm, in_=api_sb[0:1, 0:1], func=AF.Exp)

            # interleave DoubleRow and bf16 matmuls so fp8 LDWEIGHTS hide
            kseq = []
            for i in range(max(NKC8, NKCB)):
                if i < NKC8:
                    kseq.append(("dr", i))
                if i < NKCB:
                    kseq.append(("bf", i))

            # ---- main loop (scores lag 2 blocks behind the E/X pipeline
            #      so the PE instruction stream never stalls on den/u) ----
            x_hist = [None] * NBLK
            u_hist = [None] * NBLK
            for it in range(NBLK + 3):
                if it < NBLK:
                    blk = it
                    pair, j = blk // 2, blk % 2
                    if j == 0 and pair > 0:
                        e8t = enc8p.tile([P, 2 * E8C], f8, tag="enc8")
                        nc.sync.dma_start(
                            out=e8t, in_=enc8[:, pair * 2 * E8C:
                                             (pair + 1) * 2 * E8C])
                        ebt = encbp.tile([P, 2 * EBC], bf16, tag="encb")
                        nc.sync.dma_start(
                            out=ebt, in_=encbf[:, pair * 2 * EBC:
                                               (pair + 1) * 2 * EBC])
                        e8tv = e8t.rearrange(
                            "p (kc j pl n) -> p kc j pl n", kc=NKC8, j=2,
                            pl=2)
                        ebtv = ebt.rearrange(
                            "p (kc j n) -> p kc j n", kc=NKCB, j=2)
                    if pair == 0:
                        ap8 = [e80v[j][:, kc] for kc in range(NKC8)]
                        apb = [eb0v[j][:, kc] for kc in range(NKCB)]
                    else:
                        ap8 = [e8tv[:, kc, j] for kc in range(NKC8)]
                        apb = [ebtv[:, kc, j] for kc in range(NKCB)]

                    x_all = xp.tile([P, 4096], fp16, tag="x")
                    x_hist[blk] = x_all
                    last = blk == NBLK - 1
                    for mp in range(2):
                        mcs = (2 * mp, 2 * mp + 1)
                        ep_of = {}
                        for mc in mcs:
                            ep = eps.tile([P, 1024], f32, tag="e")
                            ep_of[mc] = ep
                            for half in range(2):
                                first = True
                                for kind, kx in kseq:
                                    if kind == "dr":
                                        nc.tensor.matmul(
                                            out=ep[:, half * 512:
                                                   (half + 1) * 512],
                                            lhsT=w8v[:, kx, mc],
                                            rhs=ap8[kx][:, :, half * 512:
                                                        (half + 1) * 512],
                                            start=first, stop=False,
                                            perf_mode=DR,
                                        )
                                    else:
                                        nc.tensor.matmul(
                                            out=ep[:, half * 512:
                                                   (half + 1) * 512],
                                            lhsT=w2_sb[:, kx * 512 + mc * P:
                                                       kx * 512 +
                                                       (mc + 1) * P],
                                            rhs=apb[kx][:, half * 512:
                                                        (half + 1) * 512],
                                            start=first, stop=False,
                                        )
                                    first = False
                        # the aprep/ind constants are replicated in all 4
                        # 32-row groups, so each of the 4 closers of this
                        # mc-pair can use a DISTINCT PE row group -> all four
                        # run concurrently (1 N=512 slot instead of 4)
                        for half in range(2):
                            for mc in mcs:
                                rg = (mc + 2 * half) % 4
                                nc.tensor.matmul(
                                    out=ep_of[mc][:, half * 512:
                                                  (half + 1) * 512],
                                    lhsT=api_sb[32 * rg:32 * (rg + 1),
                                                mc * P:(mc + 1) * P],
                                    rhs=api_sb[32 * rg:32 * (rg + 1),
                                               512:1024],
                                    start=False, stop=True,
                                    tile_position=(32 * rg, 0),
                                )
                        for mc in mcs:
                            if last:
                                for hf in range(2):
                                    nc.scalar.activation(
                                        out=x_all[:, mc * 1024 + hf * 512:
                                                  mc * 1024 + hf * 512 + 512],
                                        in_=ep_of[mc][:, hf * 512:
                                                      hf * 512 + 512],
                                        func=AF.Exp,
                                        scale=sinv_sb[:, mc:mc + 1],
                                    )
                            else:
                                nc.scalar.activation(
                                    out=x_all[:, mc * 1024:(mc + 1) * 1024],
                                    in_=ep_of[mc], func=AF.Exp,
                                    scale=sinv_sb[:, mc:mc + 1],
                                )

                    # den path per mc-half; fp16 reduce in/out keeps the DVE
                    # on its 2-byte fast path, tiny casts feed f32 reciprocal
                    x3 = x_all.rearrange("p (mt b) -> p mt b", b=32)
                    us = []
                    nred = 4 if last else 2
                    for hh in range(2):
                        denh = dp.tile([P, 64], fp16, tag=f"denh{hh}")
                        for q in range(nred):
                            w = 64 // nred
                            nc.vector.reduce_sum(
                                out=denh[:, q * w:(q + 1) * w],
                                in_=x3[:, hh * 64 + q * w:
                                       hh * 64 + (q + 1) * w, :],
                                axis=AX.X)
                        den = dp.tile([P, 64], f32, tag=f"den{hh}")
                        rden = dp.tile([P, 64], f32, tag=f"rden{hh}")
                        u = up.tile([P, 64], fp16, tag=f"u{hh}")
                        nsub = 2 if last else 1
                        for h2 in range(nsub):
                            w2_ = 64 // nsub
                            sl = slice(h2 * w2_, (h2 + 1) * w2_)
                            nc.vector.tensor_copy(out=den[:, sl],
                                                  in_=denh[:, sl])
                            nc.vector.reciprocal_approx_fast(
                                out=rden[:, sl], in_=den[:, sl])
                            nc.vector.tensor_mul(
                                out=u[:, sl], in0=rden[:, sl],
                                in1=vrep_sb[:, hh * 64 + h2 * w2_:
                                            hh * 64 + (h2 + 1) * w2_])
                        us.append(u)
                    u_hist[blk] = us

                # scores for pair p at it == 2p+4: all 4 col-group chains
                # (g = 2*sj + half) interleaved MM-by-MM so they run
                # CONCURRENTLY on disjoint PE column groups -- 16 matmuls in
                # ~4-5 N=512 slots instead of 16. Valid slots are
                # out[32*g + 16*half + jj, 32*jj + b].
                if it >= 4 and (it - 4) % 2 == 0 and (it - 4) // 2 < NPAIR:
                    spair = (it - 4) // 2
                    sc_ps = scps.tile([P, 512], f32, tag="sc")
                    for mc in range(4):
                        for g in range(4):
                            sj, half = divmod(g, 2)
                            sblk = 2 * spair + sj
                            nc.tensor.matmul(
                                out=sc_ps[32 * g:32 * (g + 1), :],
                                lhsT=u_hist[sblk][mc // 2][
                                    :, (mc % 2) * 32:(mc % 2) * 32 + 32],
                                rhs=x_hist[sblk][:, mc * 1024 + half * 512:
                                                mc * 1024 + half * 512 + 512],
                                start=(mc == 0), stop=(mc == 3),
                                tile_position=(0, 32 * g),
                            )
                    ssb = scb.tile([P, 512], fp16, tag="ssb")
                    nc.vector.tensor_copy(out=ssb, in_=sc_ps)
                    nc.sync.dma_start(
                        out=out[:, spair * 512:(spair + 1) * 512],
                        in_=ssb,
                    )

    nc.compile()
    return nc


def _row_scales(W2c):
    """Per-row scale minimizing fp8-e4m3 quantization MSE of the fp8 chunk."""
    import ml_dtypes
    e4 = ml_dtypes.float8_e4m3
    sc = SGRID.astype(np.float32)                        # [G]
    Wg = W2c[None, :, :] * sc[:, None, None]             # [G, H, K8]
    Qg = Wg.astype(e4).astype(np.float32) / sc[:, None, None]
    errs = ((Qg - W2c[None, :, :]) ** 2).sum(axis=2)     # [G, H]
    return sc[np.argmin(errs, axis=0)]                   # [H]


def _prep_inputs(hidden, encoder_outputs, W_attn, b_attn, v):
    """Host-side shard + layout prep. Returns in_maps for the 8 cores."""
    import ml_dtypes
    bf16 = ml_dtypes.bfloat16
    e4 = ml_dtypes.float8_e4m3

    hidden = np.asarray(hidden, dtype=np.float32)
    enc = np.asarray(encoder_outputs, dtype=np.float32)
    W = np.asarray(W_attn, dtype=np.float32)
    b = np.asarray(b_attn, dtype=np.float32)
    v = np.asarray(v, dtype=np.float32)

    K8 = NKC8 * 128
    W2 = W[:, H:]                                        # [h_out, k]
    s = _row_scales(W2[:, :K8]) if NKC8 else np.full(H, 32.0, np.float32)
    W2s = W2 * s[:, None]                                # scaled rows
    w2t = np.ascontiguousarray(W2s.T)                    # [k, h_out]

    # fp8 weights (kc8, mc, plane, m): plane0 = Q8(w), plane1 = Q8(w/16)
    w8hi = w2t[:K8].reshape(NKC8, P, 4, P).astype(e4)    # [kc, p, mc, m]
    w8lo = (w8hi.astype(np.float32) / 16.0).astype(e4)
    w8p = np.stack([w8hi, w8lo], axis=3)                 # [kc, p, mc, pl, m]
    w8p = np.ascontiguousarray(
        w8p.transpose(1, 0, 2, 3, 4).reshape(P, NKC8 * 1024))
    # bf16 weights for remaining chunks (kcb-major then m)
    w2p = np.ascontiguousarray(
        w2t[K8:].reshape(NKCB, P, H).transpose(1, 0, 2).reshape(P, NKCB * 512)
    ).astype(bf16)

    # A' = hidden @ W1.T + b_attn, exact on host, row-scaled, replicated
    apr = (hidden @ W[:, :H].T + b[None, :]) * s[None, :]  # [B, H]
    aprep = np.tile(apr, (4, 1))                         # [128, 512]
    ind = np.tile(np.eye(B, dtype=np.float32), (4, 512 // B))
    api = np.concatenate([aprep, ind], axis=1).astype(bf16)
    vcol = np.ascontiguousarray(v.reshape(4, P).T)       # [P, 4] f32
    vrep = np.repeat(vcol, 32, axis=1).astype(np.float32)
    sinv = np.ascontiguousarray(
        (1.0 / s).reshape(4, P).T.astype(np.float32))    # [P, mc]

    in_maps = []
    for c in range(NCORES):
        shard = enc[c * TC:(c + 1) * TC]                 # [TC, B, H]
        encT = np.ascontiguousarray(shard.reshape(NCOL, H).T)  # [k, NCOL]

        # fp8 chunks: hi + res planes, layout [P, pair, kc8, j, pl, 1024]
        ehi = encT[:K8].astype(e4)
        eres = ((encT[:K8] - ehi.astype(np.float32)) * 16.0).astype(e4)
        e8 = np.stack([ehi, eres], axis=1)               # [K8, pl, NCOL]
        e8 = e8.reshape(NKC8, P, 2, NPAIR, 2, 1024)      # kc p pl pr j n
        e8 = np.ascontiguousarray(
            e8.transpose(1, 3, 0, 4, 2, 5)               # p pr kc j pl n
            .reshape(P, NPAIR * NKC8 * 4096))
        # pair 0 j-major: (kc, j, pl, n) -> (j, kc, pl, n)
        c0 = NKC8 * 4096
        e8[:, :c0] = np.ascontiguousarray(
            e8[:, :c0].reshape(P, NKC8, 2, 2048).transpose(0, 2, 1, 3)
            .reshape(P, c0))

        # bf16 chunks: layout [P, pair, kcb, j, 1024]
        eb = encT[K8:].reshape(NKCB, P, NPAIR, 2, 1024)
        eb = np.ascontiguousarray(
            eb.transpose(1, 2, 0, 3, 4).reshape(P, NPAIR * NKCB * 2048))
        c0 = NKCB * 2048
        eb[:, :c0] = np.ascontiguousarray(
            eb[:, :c0].reshape(P, NKCB, 2, 1024).transpose(0, 2, 1, 3)
            .reshape(P, c0))
        in_maps.append({
            "enc8": e8, "encbf": eb.astype(bf16), "w8p": w8p, "w2p": w2p,
            "api": api, "vrep": vrep, "sinv": sinv,
        })
    return in_maps


def _assemble(results):
    """results: per-core dicts with 'scores' [128, NPAIR*512] fp16.

    Column layout: col = pair*512 + 32*jj + b. Valid rows per quarter q
    (t = 64*pair + 16*q + jj): q=0 -> row jj, q=1 -> 48+jj, q=2 -> 64+jj,
    q=3 -> 112+jj.
    """
    rowbase = (0, 48, 64, 112)
    out = np.empty((B, 1, T), np.float32)
    for c in range(NCORES):
        s = np.asarray(results[c]["scores"], dtype=np.float32)
        s4 = s.reshape(P, NPAIR, 16, B)                  # [row,pair,jj,b]
        for q in range(4):
            for jj in range(16):
                vals = s4[rowbase[q] + jj, :, jj, :]     # [pair, b]
                t0 = c * TC + 16 * q + jj
                out[:, 0, t0:t0 + 64 * NPAIR:64] = np.maximum(vals, 0.0).T
    return out


def run(in_maps, trace=False, **kw):
    from concourse.bass_utils import run_bass_kernel_spmd

    if "nc" not in _CACHE:
        _CACHE["nc"] = _build_nc()
    nc = _CACHE["nc"]
    return run_bass_kernel_spmd(
        nc, in_maps, list(range(NCORES)), trace=trace, **kw
    )


def kernel(hidden, encoder_outputs, W_attn, b_attn, v):
    in_maps = _prep_inputs(hidden, encoder_outputs, W_attn, b_attn, v)
    br = run(in_maps)
    return _assemble(br.results)
